# revision 1
# baseline (speedup 1.0000x reference)
"""LyraGemma3 sliding-window attention — Trainium2 Bass kernel, 8 NeuronCores.

Sharding: core = b*4 + h  (b in {0,1} batch, h in {0..3} head-group).
Each core owns vanilla head h, lyra head 4+h, kv head h for batch b and
produces output rows [512h, 512h+512) of batch b (the reference's
concat/transpose/reshape scramble makes those rows depend only on head h of
both streams), so the 8 cores produce disjoint slices of the final output —
no collectives.

Matmuls run as float32r (full PE rate at moving-dim >= 256, ~1.5e-4 rel err
per 128-dot vs fp32). RMS-norm's (1+w) scaling is folded into the projection
weights on the host; the rms denominator uses (1+w)^-2-weighted
sum-of-squares via a PE ones-matmul. Softmax runs without max-subtraction
(scores*scale is O(6), exp cannot overflow fp32).
"""

import sys

sys.path.insert(0, "/opt/trn_rl_repo")

import numpy as np

import concourse.bass as bass
import concourse.tile as tile
from concourse import mybir
from concourse.tile import ScopedClock

F32 = mybir.dt.float32
F32R = mybir.dt.float32r
AF = mybir.ActivationFunctionType

B, S, HID = 2, 2048, 2560
H, KV, D = 8, 4, 256
WINDOW = 1024
THETA = 10000.0
EPS = 1e-6
SCALING = 256.0 ** (-0.5)  # 1/16

NKC = HID // 128  # 20 contraction chunks for projections
NST = 8           # phase-A s-tiles of 256 tokens
NT = S // 128     # 16 key tiles of 128
NQ = 4            # attention q-tiles of 512
MASK_NEG = -1e30


class SplitWaitTC(tile.TileContext):
    """This container's walrus encodes at most ONE semaphore wait per
    instruction; Tile emits multi-wait sync_info. Hoist extra waits onto
    preceding same-engine NOPs."""

    def _drain_and_barrier(self, tick_clock, wait_clock):
        nc = self.nc
        drain_inst = nc.sync.drain()
        wait_clock.add_sem_waits(
            drain_inst.ins, ScopedClock({None: tick_clock.global_clock})
        )
        self._split_multi_waits()
        nc.all_engine_barrier()
        popped = nc._tile_sem_poison_stack.pop()
        assert popped is self._sem_poison
        nc.clear_and_free_semaphores(list(self.sems.allocated().values()))
        nc.all_engine_barrier()

    def _split_multi_waits(self):
        nc = self.nc
        cur_bb = nc.cur_bb
        assert cur_bb is not None
        for f in nc.m.functions:
            for blk in f.blocks:
                insts = blk.instructions
                i = 0
                while i < len(insts):
                    inst = insts[i]
                    si = inst.sync_info
                    if si is not None and si.on_wait and len(si.on_wait) > 1:
                        waits = list(si.on_wait)
                        inst.sync_info = mybir.SyncInfo(
                            on_wait=waits[-1:], on_update=si.on_update
                        )
                        eng = inst.engine
                        for w in waits[:-1]:
                            nop = nc.engines[eng].nop()
                            nop.ins.sync_info = mybir.SyncInfo(
                                on_wait=[w], on_update=[]
                            )
                            cur_bb.bb.instructions.remove(nop.ins)
                            insts.insert(i, nop.ins)
                            i += 1
                    i += 1


def _mask_index(T, Q):
    """Mask tile for key-tile T against q-tile Q (queries [512Q,512Q+512)).
    Returns None (fully valid), 4+j (causal), or j'' (window edge)."""
    j = T - 4 * Q
    if j >= 0:
        return 4 + j
    if T >= 4 * Q - 4:
        return None
    return T - (4 * Q - 8)


def build_program():
    nc = bass.Bass()

    hsT = nc.declare_dram_parameter("hsT", [HID, S], F32R, isOutput=False)
    wq2 = nc.declare_dram_parameter("wq2", [HID, 512], F32R, isOutput=False)
    wk1 = nc.declare_dram_parameter("wk1", [HID, 256], F32R, isOutput=False)
    wv1 = nc.declare_dram_parameter("wv1", [HID, 256], F32R, isOutput=False)
    wo_d = nc.declare_dram_parameter("wo", [H * D, HID], F32R, isOutput=False)
    cos_d = nc.declare_dram_parameter("cos_t", [128, S], F32, isOutput=False)
    sin_d = nc.declare_dram_parameter("sin_t", [128, S], F32, isOutput=False)
    masks_d = nc.declare_dram_parameter("masks", [8 * 128, 512], F32, isOutput=False)
    invq_d = nc.declare_dram_parameter("invq", [128, 2], F32R, isOutput=False)
    invk_d = nc.declare_dram_parameter("invk", [128, 2], F32R, isOutput=False)
    onec_d = nc.declare_dram_parameter("onec", [128, 1], F32R, isOutput=False)
    oner_d = nc.declare_dram_parameter("oner", [1, 128], F32R, isOutput=False)
    epsb_d = nc.declare_dram_parameter("epsb", [1, 1], F32, isOutput=False)
    out_d = nc.declare_dram_parameter("out", [512, HID], F32, isOutput=True)

    # DRAM spill for phase-A results (re-loaded in phase C)
    qT_sp = nc.dram_tensor("qT_sp", [4, 128, S], F32R)
    kTn_sp = nc.dram_tensor("kTn_sp", [2, 128, S], F32R)
    kTr_sp = nc.dram_tensor("kTr_sp", [2, 128, S], F32R)
    v_sp = nc.dram_tensor("v_sp", [128, NT * 256], F32R)

    with SplitWaitTC(nc) as tc:
        with (
            tc.tile_pool(name="outer", bufs=1) as pO,
            tc.tile_pool(name="outerps", bufs=1, space="PSUM") as _psO,
        ):
            onec = pO.tile([128, 1], F32R, name="onec")
            nc.sync.dma_start(onec[:], onec_d[:])
            oner = pO.tile([1, 128], F32R, name="oner")
            nc.sync.dma_start(oner[:], oner_d[:])
            epsb = pO.tile([1, 1], F32, name="epsb")
            nc.sync.dma_start(epsb[:], epsb_d[:])
            # ================= PHASE A: projections + norm + rope ========
            with (
                tc.tile_pool(name="pAw", bufs=1) as pW,
                tc.tile_pool(name="pA", bufs=1) as pA,
                tc.tile_pool(name="pAps", bufs=1, space="PSUM") as psA,
            ):
                wq_sb = pW.tile([128, NKC * 512], F32R, name="wq_sb")
                nc.sync.dma_start(
                    wq_sb[:].rearrange("p (c d) -> p c d", c=NKC),
                    wq2[:].rearrange("(c p) d -> c p d", p=128).transpose([1, 0, 2]),
                )
                wk_sb = pW.tile([128, NKC * 256], F32R, name="wk_sb")
                nc.sync.dma_start(
                    wk_sb[:].rearrange("p (c d) -> p c d", c=NKC),
                    wk1[:].rearrange("(c p) d -> c p d", p=128).transpose([1, 0, 2]),
                )
                wv_sb = pW.tile([128, NKC * 256], F32R, name="wv_sb")
                nc.sync.dma_start(
                    wv_sb[:].rearrange("p (c d) -> p c d", c=NKC),
                    wv1[:].rearrange("(c p) d -> c p d", p=128).transpose([1, 0, 2]),
                )
                cos_sb = pW.tile([128, S], F32, name="cos_sb")
                nc.sync.dma_start(cos_sb[:], cos_d[:])
                sin_sb = pW.tile([128, S], F32, name="sin_sb")
                nc.sync.dma_start(sin_sb[:], sin_d[:])
                invq = pW.tile([128, 2], F32R, name="invq")
                nc.sync.dma_start(invq[:], invq_d[:])
                invk = pW.tile([128, 2], F32R, name="invk")
                nc.sync.dma_start(invk[:], invk_d[:])

                for st in range(NST):
                    s0 = st * 256
                    hst = pA.tile([128, NKC * 256], F32R, name="hst", bufs=2)
                    nc.sync.dma_start(
                        hst[:].rearrange("p (c s) -> p c s", c=NKC),
                        hsT[:, s0 : s0 + 256]
                        .rearrange("(c p) s -> c p s", p=128)
                        .transpose([1, 0, 2]),
                    )
                    # ---- projections (accumulate over 20 HID chunks) ----
                    qz = pA.tile([128, 1024], F32, name="qz", bufs=2)
                    for hc in range(4):
                        pq = psA.tile([128, 256], F32, name="pacc", bufs=3)
                        for kc in range(NKC):
                            nc.tensor.matmul(
                                pq[:],
                                wq_sb[:, kc * 512 + hc * 128 : kc * 512 + (hc + 1) * 128],
                                hst[:, kc * 256 : (kc + 1) * 256],
                                start=(kc == 0),
                                stop=(kc == NKC - 1),
                            )
                        nc.vector.tensor_copy(qz[:, hc * 256 : (hc + 1) * 256], pq[:])
                    kz = pA.tile([128, 512], F32, name="kz", bufs=2)
                    for hc in range(2):
                        pk = psA.tile([128, 256], F32, name="pacc", bufs=3)
                        for kc in range(NKC):
                            nc.tensor.matmul(
                                pk[:],
                                wk_sb[:, kc * 256 + hc * 128 : kc * 256 + (hc + 1) * 128],
                                hst[:, kc * 256 : (kc + 1) * 256],
                                start=(kc == 0),
                                stop=(kc == NKC - 1),
                            )
                        nc.vector.tensor_copy(kz[:, hc * 256 : (hc + 1) * 256], pk[:])
                    vst = pA.tile([128, 512], F32R, name="vst", bufs=2)
                    for sm in range(2):
                        pv = psA.tile([128, 256], F32, name="pacc", bufs=3)
                        for kc in range(NKC):
                            nc.tensor.matmul(
                                pv[:],
                                hst[:, kc * 256 + sm * 128 : kc * 256 + sm * 128 + 128],
                                wv_sb[:, kc * 256 : (kc + 1) * 256],
                                start=(kc == 0),
                                stop=(kc == NKC - 1),
                            )
                        nc.vector.tensor_copy(vst[:, sm * 256 : (sm + 1) * 256], pv[:])
                    nc.sync.dma_start(v_sp[:, st * 512 : (st + 1) * 512], vst[:])

                    # ---- rms-norm factors (3 heads: qv, ql, k) ----
                    sqq = pA.tile([128, 1024], F32R, name="sqq", bufs=2)
                    nc.scalar.activation(sqq[:], qz[:], AF.Square)
                    sqk = pA.tile([128, 512], F32R, name="sqk", bufs=2)
                    nc.scalar.activation(sqk[:], kz[:], AF.Square)
                    bcs = []
                    for head in range(3):  # 0: q vanilla, 1: q lyra, 2: k
                        pn = psA.tile([1, 256], F32, name="pn", bufs=2)
                        for c in range(2):
                            if head < 2:
                                rhs = sqq[:, (head * 2 + c) * 256 : (head * 2 + c + 1) * 256]
                                lhsT = invq[:, c : c + 1]
                            else:
                                rhs = sqk[:, c * 256 : (c + 1) * 256]
                                lhsT = invk[:, c : c + 1]
                            nc.tensor.matmul(
                                pn[:], lhsT, rhs, start=(c == 0), stop=(c == 1)
                            )
                        srt = pA.tile([1, 256], F32, name="srt", bufs=2)
                        nc.scalar.activation(
                            srt[:], pn[:], AF.Sqrt, bias=epsb[:], scale=1.0 / 256.0
                        )
                        rst = pA.tile([1, 256], F32R, name="rst", bufs=2)
                        with nc.allow_low_precision(reason="rms rstd"):
                            nc.vector.reciprocal(rst[:], srt[:])
                        pbc = psA.tile([128, 256], F32, name="pbc", bufs=2)
                        nc.tensor.matmul(pbc[:], oner[:], rst[:], start=True, stop=True)
                        bc = pA.tile([128, 256], F32, name=f"bc{head}", bufs=2)
                        nc.vector.tensor_copy(bc[:], pbc[:])
                        bcs.append(bc)

                    # ---- rope + apply rstd ----
                    cs = cos_sb[:, s0 : s0 + 256]
                    sn = sin_sb[:, s0 : s0 + 256]

                    def rope2(z0, z1, bc, d0, d1):
                        t0 = pA.tile([128, 256], F32, name="t0", bufs=2)
                        nc.vector.tensor_mul(t0[:], z0, cs)
                        t1 = pA.tile([128, 256], F32, name="t1", bufs=2)
                        nc.vector.tensor_mul(t1[:], z1, sn)
                        u0 = pA.tile([128, 256], F32, name="u0", bufs=2)
                        nc.vector.tensor_sub(u0[:], t0[:], t1[:])
                        nc.vector.tensor_mul(d0, u0[:], bc[:])
                        t2 = pA.tile([128, 256], F32, name="t2", bufs=2)
                        nc.vector.tensor_mul(t2[:], z1, cs)
                        t3 = pA.tile([128, 256], F32, name="t3", bufs=2)
                        nc.vector.tensor_mul(t3[:], z0, sn)
                        u1 = pA.tile([128, 256], F32, name="u1", bufs=2)
                        nc.vector.tensor_add(u1[:], t2[:], t3[:])
                        nc.vector.tensor_mul(d1, u1[:], bc[:])

                    qro = pA.tile([128, 1024], F32R, name="qro", bufs=2)
                    for head in range(2):
                        rope2(
                            qz[:, (head * 2) * 256 : (head * 2) * 256 + 256],
                            qz[:, (head * 2 + 1) * 256 : (head * 2 + 1) * 256 + 256],
                            bcs[head],
                            qro[:, (head * 2) * 256 : (head * 2) * 256 + 256],
                            qro[:, (head * 2 + 1) * 256 : (head * 2 + 1) * 256 + 256],
                        )
                    krst = pA.tile([128, 512], F32R, name="krst", bufs=2)
                    rope2(
                        kz[:, 0:256], kz[:, 256:512], bcs[2],
                        krst[:, 0:256], krst[:, 256:512],
                    )
                    knst = pA.tile([128, 512], F32R, name="knst", bufs=2)
                    nc.vector.tensor_mul(knst[:, 0:256], kz[:, 0:256], bcs[2][:])
                    nc.vector.tensor_mul(knst[:, 256:512], kz[:, 256:512], bcs[2][:])

                    nc.sync.dma_start(
                        qT_sp[:, :, s0 : s0 + 256].transpose([1, 0, 2]),
                        qro[:].rearrange("p (c s) -> p c s", c=4),
                    )
                    nc.sync.dma_start(
                        kTr_sp[:, :, s0 : s0 + 256].transpose([1, 0, 2]),
                        krst[:].rearrange("p (c s) -> p c s", c=2),
                    )
                    nc.sync.dma_start(
                        kTn_sp[:, :, s0 : s0 + 256].transpose([1, 0, 2]),
                        knst[:].rearrange("p (c s) -> p c s", c=2),
                    )

            # ================= PHASES C+D ================================
            with tc.tile_pool(name="pOC", bufs=1) as pOC:
              # normalized attention outputs in combinedT (wo-lhsT) layout:
              # outC[stream][dc][:, j*256 + m] = outT[stream][dc][d, 8m+j]
              outC = [
                  [pOC.tile([128, S], F32R, name=f"outC{s}{c}") for c in range(2)]
                  for s in range(2)
              ]
              # ================= PHASE C: attention ========================
              with (
                tc.tile_pool(name="pCk", bufs=1) as pK,
                  tc.tile_pool(name="pC", bufs=1) as pC,
                  tc.tile_pool(name="pCps", bufs=1, space="PSUM") as psC,
              ):
                  masks_sb = pK.tile([128, 8 * 512], F32, name="masks_sb")
                  nc.sync.dma_start(
                      masks_sb[:].rearrange("p (m s) -> p m s", m=8),
                      masks_d[:].rearrange("(m p) s -> m p s", p=128).transpose([1, 0, 2]),
                  )
                  kTr_all = pK.tile([128, 2 * S], F32R, name="kTr_all")
                  nc.sync.dma_start(
                      kTr_all[:].rearrange("p (c s) -> p c s", c=2),
                      kTr_sp[:, :, :].transpose([1, 0, 2]),
                  )
                  kTn_all = pK.tile([128, 2 * S], F32R, name="kTn_all")
                  nc.sync.dma_start(
                      kTn_all[:].rearrange("p (c s) -> p c s", c=2),
                      kTn_sp[:, :, :].transpose([1, 0, 2]),
                  )
                  v_all = pK.tile([128, NT * 256], F32R, name="v_all")
                  nc.sync.dma_start(v_all[:], v_sp[:, :])

                  for stream in range(2):  # 0 = vanilla (roped k), 1 = lyra
                      kT = kTr_all if stream == 0 else kTn_all
                      for Q in range(NQ):
                          qTq = pC.tile([128, 1024], F32R, name="qTq", bufs=2)
                          nc.sync.dma_start(
                              qTq[:].rearrange("p (c s) -> p c s", c=2),
                              qT_sp[
                                  2 * stream : 2 * stream + 2, :, Q * 512 : (Q + 1) * 512
                              ].transpose([1, 0, 2]),
                          )
                          T_lo = max(0, 4 * Q - 8)
                          T_hi = 4 * Q + 3
                          po0 = psC.tile([128, 512], F32, name="po0", bufs=1)
                          po1 = psC.tile([128, 512], F32, name="po1", bufs=1)
                          psm = psC.tile([1, 512], F32, name="psm", bufs=1)
                          for T in range(T_lo, T_hi + 1):
                              pss = psC.tile([128, 512], F32, name="pss", bufs=2)
                              nc.tensor.matmul(
                                  pss[:],
                                  kT[:, T * 128 : (T + 1) * 128],
                                  qTq[:, 0:512],
                                  start=True,
                                  stop=False,
                              )
                              nc.tensor.matmul(
                                  pss[:],
                                  kT[:, S + T * 128 : S + (T + 1) * 128],
                                  qTq[:, 512:1024],
                                  start=False,
                                  stop=True,
                              )
                              midx = _mask_index(T, Q)
                              probs = pC.tile([128, 512], F32R, name="probs", bufs=3)
                              if midx is None:
                                  nc.scalar.activation(
                                      probs[:], pss[:], AF.Exp, scale=SCALING
                                  )
                              else:
                                  sct = pC.tile([128, 512], F32, name="sct", bufs=2)
                                  nc.vector.tensor_add(
                                      sct[:],
                                      pss[:],
                                      masks_sb[:, midx * 512 : (midx + 1) * 512],
                                  )
                                  nc.scalar.activation(
                                      probs[:], sct[:], AF.Exp, scale=SCALING
                                  )
                              first = T == T_lo
                              last = T == T_hi
                              nc.tensor.matmul(
                                  psm[:], onec[:], probs[:], start=first, stop=last
                              )
                              nc.tensor.matmul(
                                  po0[:],
                                  v_all[:, T * 256 : T * 256 + 128],
                                  probs[:],
                                  start=first,
                                  stop=last,
                              )
                              nc.tensor.matmul(
                                  po1[:],
                                  v_all[:, T * 256 + 128 : T * 256 + 256],
                                  probs[:],
                                  start=first,
                                  stop=last,
                              )
                          rstC = pC.tile([1, 512], F32R, name="rstC", bufs=2)
                          with nc.allow_low_precision(reason="softmax denom"):
                              nc.vector.reciprocal(rstC[:], psm[:])
                          pbcC = psC.tile([128, 512], F32, name="pbcC", bufs=1)
                          nc.tensor.matmul(pbcC[:], oner[:], rstC[:], start=True, stop=True)
                          bcsC = pC.tile([128, 512], F32, name="bcsC", bufs=2)
                          nc.vector.tensor_copy(bcsC[:], pbcC[:])
                          for dc in range(2):
                              po = po0 if dc == 0 else po1
                              in_ap = po[:].rearrange("p (m j) -> p m j", j=8)
                              bc_ap = bcsC[:].rearrange("p (m j) -> p m j", j=8)
                              out_ap = (
                                  outC[stream][dc][:]
                                  .rearrange("p (j m) -> p j m", j=8)
                                  .transpose([0, 2, 1])[:, Q * 64 : (Q + 1) * 64, :]
                              )
                              nc.vector.tensor_mul(out_ap, in_ap, bc_ap)

              # ================= PHASE D: output projection ================
              with (
                  tc.tile_pool(name="pD", bufs=1) as pD,
                  tc.tile_pool(name="pDps", bufs=1, space="PSUM") as psD,
              ):
                  for co, w in ((0, 1024), (1024, 1024), (2048, 512)):
                      ntiles = w // 512
                      pos = [
                          [
                              psD.tile([128, 512], F32, name=f"pD{m}{nt}", bufs=1)
                              for nt in range(ntiles)
                          ]
                          for m in range(4)
                      ]
                      for kc in range(16):
                          wosb = pD.tile([128, 1024], F32R, name="wosb", bufs=3)
                          nc.sync.dma_start(
                              wosb[:, 0:w], wo_d[kc * 128 : (kc + 1) * 128, co : co + w]
                          )
                          j, dc = kc // 2, kc % 2
                          for m in range(4):
                              stream, m0 = m // 2, (m % 2) * 128
                              lhsT = outC[stream][dc][:, j * 256 + m0 : j * 256 + m0 + 128]
                              for nt in range(ntiles):
                                  nc.tensor.matmul(
                                      pos[m][nt][:],
                                      lhsT,
                                      wosb[:, nt * 512 : (nt + 1) * 512],
                                      start=(kc == 0),
                                      stop=(kc == 15),
                                  )
                      for m in range(4):
                          for nt in range(ntiles):
                              ost = pD.tile([128, 512], F32, name="ost", bufs=3)
                              nc.vector.tensor_copy(ost[:], pos[m][nt][:])
                              nc.sync.dma_start(
                                  out_d[
                                      m * 128 : (m + 1) * 128,
                                      co + nt * 512 : co + (nt + 1) * 512,
                                  ],
                                  ost[:],
                              )
    return nc


def _host_inputs(hidden_states, wq, wk, wv, wo, q_norm_w, k_norm_w):
    """Build the 8 per-core input maps (all host-side numpy prep)."""
    hs = np.asarray(hidden_states, dtype=np.float32)
    wq = np.asarray(wq, dtype=np.float32)
    wk = np.asarray(wk, dtype=np.float32)
    wv = np.asarray(wv, dtype=np.float32)
    wo = np.ascontiguousarray(np.asarray(wo, dtype=np.float32))
    qnw = np.asarray(q_norm_w, dtype=np.float32)
    knw = np.asarray(k_norm_w, dtype=np.float32)

    hsT = [np.ascontiguousarray(hs[b].T) for b in range(B)]

    inv_freq = 1.0 / (THETA ** (np.arange(0, D, 2, dtype=np.float32) / D))
    ang = np.outer(inv_freq, np.arange(S, dtype=np.float32))  # (128, S)
    cos_t = np.ascontiguousarray(np.cos(ang), dtype=np.float32)
    sin_t = np.ascontiguousarray(np.sin(ang), dtype=np.float32)

    x = np.arange(128)[:, None]
    y = np.arange(512)[None, :]
    masks = np.empty((8, 128, 512), np.float32)
    for jj in range(4):  # window-edge: valid iff y < x + 128*jj
        masks[jj] = np.where(y < x + 128 * jj, 0.0, MASK_NEG)
    for j in range(4):  # causal: valid iff y >= x + 128*j
        masks[4 + j] = np.where(y >= x + 128 * j, 0.0, MASK_NEG)
    masks = np.ascontiguousarray(masks.reshape(8 * 128, 512))

    invq = np.ascontiguousarray(
        ((1.0 + qnw) ** -2).reshape(2, 128).T, dtype=np.float32
    )
    invk = np.ascontiguousarray(
        ((1.0 + knw) ** -2).reshape(2, 128).T, dtype=np.float32
    )
    onec = np.ones((128, 1), np.float32)
    oner = np.ones((1, 128), np.float32)

    qs = 1.0 + qnw
    ks = 1.0 + knw
    in_maps = []
    for core in range(8):
        b, h = core // 4, core % 4
        wq2 = np.concatenate(
            [
                wq[:, h * D : (h + 1) * D] * qs[None, :],
                wq[:, (4 + h) * D : (5 + h) * D] * qs[None, :],
            ],
            axis=1,
        )
        in_maps.append(
            {
                "hsT": hsT[b],
                "wq2": np.ascontiguousarray(wq2),
                "wk1": np.ascontiguousarray(wk[:, h * D : (h + 1) * D] * ks[None, :]),
                "wv1": np.ascontiguousarray(wv[:, h * D : (h + 1) * D]),
                "wo": wo,
                "cos_t": cos_t,
                "sin_t": sin_t,
                "masks": masks,
                "invq": invq,
                "invk": invk,
                "onec": onec,
                "epsb": np.full((1, 1), EPS, np.float32),
                "oner": oner,
            }
        )
    return in_maps


_PROGRAM = None


def kernel(hidden_states, wq, wk, wv, wo, q_norm_w, k_norm_w):
    global _PROGRAM
    from concourse.bass_utils import run_bass_kernel_spmd

    if _PROGRAM is None:
        _PROGRAM = build_program()
    in_maps = _host_inputs(hidden_states, wq, wk, wv, wo, q_norm_w, k_norm_w)
    res = run_bass_kernel_spmd(_PROGRAM, in_maps, core_ids=list(range(8)))
    out = np.empty((B, S, HID), np.float32)
    for core in range(8):
        b, h = core // 4, core % 4
        out[b, h * 512 : (h + 1) * 512, :] = res.results[core]["out"]
    return out

